# revision 7
# baseline (speedup 1.0000x reference)
"""Trainium2 Bass kernel for causal multi-head attention.

Problem: x[4, 2048, 1024] -> Attention(heads=16, causal) -> out[4, 2048, 1024]

Sharding over 8 NeuronCores: core c handles batch bi = c // 2 and head-half
hj = c % 2 (8 of the 16 heads).  Each core computes its 8 heads' attention
and a partial output projection (row-parallel Wo); the host sums the two
partials per batch element and adds bo (the all-reduce step).

Per-core kernel (n=2048 tokens, dloc=512 local features, dh=64, 8 heads):
  - Host supplies x^T (so the contraction dim lands on SBUF partitions with
    no on-device transposes).
  - Q^T, K^T [128 feats(pair of heads), 2048] per head-pair; V [2048, 512]
    in natural layout with a ones-column per head (V' = [V | 1]) so the
    PV matmul accumulates softmax denominators for free.
  - Scores are computed transposed: S^T[j, i] = k_j . q_i with K^T as the
    stationary operand (K=dh=64, two heads packed onto PE row-groups 0-1 /
    2-3 which the hardware runs concurrently).
  - exp on ScalarE straight out of PSUM (scale=1/8 fused); causal mask via
    gpsimd affine_select (fill 0) on the diagonal tiles only.
  - O'^T[f, i] accumulated over j-tiles in PSUM via lhsT=V'; row 64 is the
    softmax denominator.  Normalize = DVE reciprocal + a K=1 outer-product
    matmul to broadcast 1/sum down partitions + DVE multiply (+bv).
  - Output projection contracts the 512 local features from O^T directly.

All matmul operands are bitcast to float32r (fp32 truncated to fp22 inside
the PE) which streams at 1 cycle/row for free dims >= 256 - bf16 speed with
~6e-5 relative error.
"""

import os
import sys

for _p in ("/opt/trn_rl_repo",):
    if _p not in sys.path and os.path.isdir(_p):
        sys.path.insert(0, _p)

import numpy as np

import concourse.bass as bass
import concourse.mybir as mybir
import concourse.tile as tile
from concourse import bacc
from concourse import bass_utils

ts = bass.ts
F32 = mybir.dt.float32
F32R = mybir.dt.float32r

P = 128          # SBUF partitions
N = 2048         # sequence length
D = 1024         # model dim
DLOC = 512       # local (per-core) feature dim = 8 heads * 64
DH = 64          # head dim
NPAIR = 4        # head pairs per core (2 heads per pair = 128 feats)
NCO = D // P     # 8 contraction tiles over model dim
NJT = N // P     # 16 key tiles of 128
NCH = N // 512   # 4 query chunks of 512
SCALE = DH ** -0.5


def _emit_kernel(tc, xT_d, wq_d, wk_d, wv_d, wo_d, bq_d, bk_d, bv_d, ones_d, out_d):
    nc = tc.nc
    EXP = mybir.ActivationFunctionType.Exp
    ADD = mybir.AluOpType.add
    GE = mybir.AluOpType.is_ge

    xTr = xT_d.rearrange("(o p) t -> p o t", p=P)
    wqr = wq_d.rearrange("(o p) f -> p o f", p=P)
    wkr = wk_d.rearrange("(o p) f -> p o f", p=P)
    wvr = wv_d.rearrange("(o p) f -> p o f", p=P)
    wor = wo_d.rearrange("(o p) e -> p o e", p=P)

    with (
        nc.allow_low_precision(reason="fp32r (fp22) compute is within tolerance"),
        tc.tile_pool(name="perm", bufs=1) as perm,
        tc.tile_pool(name="shared", bufs=1) as shared,
        tc.tile_pool(name="qkt", bufs=2) as qktp,
        tc.tile_pool(name="pexp", bufs=2) as pexp,
        tc.tile_pool(name="rrp", bufs=1) as rrp,
        tc.tile_pool(name="outsb", bufs=1) as outsb,
        tc.tile_pool(name="psS", bufs=2, space="PSUM") as psS,
        tc.tile_pool(name="psO", bufs=1, space="PSUM") as psO,
        tc.tile_pool(name="psProj", bufs=1, space="PSUM") as psProj,
        tc.tile_pool(name="psR", bufs=1, space="PSUM") as psR,
    ):
        # ---- constants / weights ----------------------------------------
        bq_sb = perm.tile([P, NPAIR], F32, name="bq_sb")
        bk_sb = perm.tile([P, NPAIR], F32, name="bk_sb")
        bv_sb = perm.tile([P, NPAIR], F32, name="bv_sb")
        nc.sync.dma_start(out=bq_sb, in_=bq_d)
        nc.sync.dma_start(out=bk_sb, in_=bk_d)
        nc.sync.dma_start(out=bv_sb, in_=bv_d)

        ones_sb = perm.tile([P, DH], F32R, name="ones_sb")
        nc.sync.dma_start(out=ones_sb, in_=ones_d)

        # V' = [V | 1] per head: [128 j, jt, head, 65]; ones column via a
        # broadcast DMA from the ones input (f32r memset fails ISA codegen)
        Vp = perm.tile([P, NJT, 8, DH + 1], F32R, name="Vp")
        ones_bcast = bass.AP(
            tensor=ones_d.tensor,
            offset=0,
            ap=[[DH, P], [0, NJT * 8], [1, 1]],
        )
        nc.sync.dma_start(out=Vp[:, :, :, DH:], in_=ones_bcast)

        xT_sb = perm.tile([P, NCO, N], F32R, name="xT_sb")
        for ch in range(NCH):
            for co in range(NCO):
                nc.sync.dma_start(
                    out=xT_sb[:, co, ts(ch, 512)], in_=xTr[:, co, ts(ch, 512)]
                )
        wq_sb = shared.tile([P, NCO, DLOC], F32R, name="wq_sb", tag="wq_wo")
        wk_sb = shared.tile([P, NCO, DLOC], F32R, name="wk_sb", tag="wk")
        wv_sb = shared.tile([P, NCO, DLOC], F32R, name="wv_sb", tag="wv_ot")
        for co in range(NCO):
            nc.sync.dma_start(out=wq_sb[:, co, :], in_=wqr[:, co, :])
        for co in range(NCO):
            nc.sync.dma_start(out=wk_sb[:, co, :], in_=wkr[:, co, :])
        for co in range(NCO):
            nc.sync.dma_start(out=wv_sb[:, co, :], in_=wvr[:, co, :])

        qk_tiles = {}

        def qkproj_gen(pair, use_big_psum):
            """Emit Q^T / K^T projection for one head pair; yields between ops."""
            QT = qktp.tile([P, N], F32R, name=f"QT{pair}", tag="qt")
            KT = qktp.tile([P, N], F32R, name=f"KT{pair}", tag="kt")
            qk_tiles[pair] = (QT, KT)
            for wsb, dst, bias in ((wq_sb, QT, bq_sb), (wk_sb, KT, bk_sb)):
                for ch in range(NCH):
                    if use_big_psum:
                        grp = psS.tile([P, 2, 512], F32, name="pj", tag="sg")
                        acc = grp[:, 0, :]
                    else:
                        acc = psProj.tile([P, 512], F32, name="pj", tag="proj")
                    for co in range(NCO):
                        nc.tensor.matmul(
                            acc,
                            lhsT=wsb[:, co, ts(pair, P)],
                            rhs=xT_sb[:, co, ts(ch, 512)],
                            start=(co == 0),
                            stop=(co == NCO - 1),
                        )
                        yield
                    nc.vector.tensor_scalar_add(
                        out=dst[:, ts(ch, 512)],
                        in0=acc,
                        scalar1=bias[:, pair : pair + 1],
                    )
                    yield

        def vproj_emit():
            for jt in range(NJT):
                grp = psS.tile([P, 2, 512], F32, name="vps", tag="sg")
                acc = grp[:, 0, :]
                for co in range(NCO):
                    nc.tensor.matmul(
                        acc,
                        lhsT=xT_sb[:, co, ts(jt, P)],
                        rhs=wv_sb[:, co, :],
                        start=(co == 0),
                        stop=(co == NCO - 1),
                    )
                nc.vector.tensor_copy(
                    out=Vp[:, jt, :, 0:DH],
                    in_=acc.rearrange("p (h f) -> p h f", h=8),
                )

        def attn_emit(pair, fill):
            QT, KT = qk_tiles[pair]
            hA, hB = 2 * pair, 2 * pair + 1
            for ch in range(NCH):
                oA = psO.tile([P, 512], F32, name="oA", tag="oA")
                oB = psO.tile([P, 512], F32, name="oB", tag="oB")
                njt = 4 * ch + 4
                prev = None

                def pv(pt, jt, njt=njt, oA=oA, oB=oB, hA=hA, hB=hB):
                    for h01, (oP, h) in enumerate(((oA, hA), (oB, hB))):
                        nc.tensor.matmul(
                            oP[0 : DH + 1, :],
                            lhsT=Vp[:, jt, h, :],
                            rhs=pt[:, h01, :],
                            start=(jt == 0),
                            stop=(jt == njt - 1),
                        )

                for jt in range(njt):
                    sg = psS.tile([P, 2, 512], F32, name="sg", tag="sg")
                    nc.tensor.matmul(
                        sg[:, 0, :],
                        lhsT=KT[0:DH, ts(jt, P)],
                        rhs=QT[0:DH, ts(ch, 512)],
                        start=True,
                        stop=True,
                    )
                    nc.tensor.matmul(
                        sg[:, 1, :],
                        lhsT=KT[DH:P, ts(jt, P)],
                        rhs=QT[DH:P, ts(ch, 512)],
                        start=True,
                        stop=True,
                    )
                    pt = pexp.tile([P, 2, 512], F32R, name="pt", tag="pt")
                    nc.scalar.activation(out=pt, in_=sg, func=EXP, scale=SCALE)
                    r = jt - 4 * ch
                    if r >= 0:
                        w = P * (r + 1)
                        for h01 in (0, 1):
                            # keep where q >= 128*r + p  (j_global <= i_global)
                            nc.gpsimd.affine_select(
                                out=pt[:, h01, 0:w],
                                in_=pt[:, h01, 0:w],
                                compare_op=GE,
                                fill=0.0,
                                base=-P * r,
                                channel_multiplier=-1,
                                pattern=[[1, w]],
                            )
                    if prev is not None:
                        pv(*prev)
                    prev = (pt, jt)
                    if fill is not None:
                        for _ in range(2):
                            if next(fill, StopIteration) is StopIteration:
                                fill = None
                                break
                pv(*prev)

                # ---- normalize: O = O' * (1/rowsum), + bv -----------------
                rR = rrp.tile([P, 2, 512], F32R, name="rR", tag="rR")
                nc.vector.reciprocal(out=rR[DH : DH + 1, 0, :], in_=oA[DH : DH + 1, :])
                nc.vector.reciprocal(out=rR[DH : DH + 1, 1, :], in_=oB[DH : DH + 1, :])
                for h01 in (0, 1):
                    Rp = psR.tile([DH, 512], F32, name="Rp", tag="Rp")
                    nc.tensor.matmul(
                        Rp,
                        lhsT=ones_sb[DH : DH + 1, 0:DH],
                        rhs=rR[DH : DH + 1, h01, :],
                        start=True,
                        stop=True,
                    )
                    nc.vector.tensor_copy(out=rR[0:DH, h01, :], in_=Rp)
                nc.vector.tensor_mul(
                    out=OT[0:DH, pair, ts(ch, 512)], in0=oA[0:DH, :], in1=rR[0:DH, 0, :]
                )
                nc.vector.tensor_scalar_add(
                    out=OT[0:DH, pair, ts(ch, 512)],
                    in0=OT[0:DH, pair, ts(ch, 512)],
                    scalar1=bv_sb[0:DH, pair : pair + 1],
                )
                nc.vector.tensor_mul(
                    out=OT[DH:P, pair, ts(ch, 512)], in0=oB[0:DH, :], in1=rR[0:DH, 1, :]
                )
                nc.vector.tensor_scalar_add(
                    out=OT[DH:P, pair, ts(ch, 512)],
                    in0=OT[DH:P, pair, ts(ch, 512)],
                    scalar1=bv_sb[DH:P, pair : pair + 1],
                )

        def outproj_chunk(ch):
            for it in range(4 * ch, 4 * ch + 4):
                for e in range(2):
                    acc = psProj.tile([P, 512], F32, name="ops", tag="proj")
                    for p4 in range(NPAIR):
                        nc.tensor.matmul(
                            acc,
                            lhsT=OT[:, p4, ts(it, P)],
                            rhs=wo_sb[:, p4, ts(e, 512)],
                            start=(p4 == 0),
                            stop=(p4 == NPAIR - 1),
                        )
                    ob = outsb.tile([P, 512], F32, name="ob", tag="ob")
                    nc.vector.tensor_copy(out=ob, in_=acc)
                    nc.sync.dma_start(out=out_d[ts(it, P), ts(e, 512)], in_=ob)

        # ---- emission schedule ------------------------------------------
        # upfront: QK projections for pair 0 + all of V (PE dense, ACT idle)
        for _ in qkproj_gen(0, use_big_psum=True):
            pass
        vproj_emit()

        # OT reuses wv's slot (wv is dead once V is projected)
        OT = shared.tile([P, NPAIR, N], F32R, name="OT", tag="wv_ot")

        # attention for pair p overlapped with projections for pair p+1
        attn_emit(0, qkproj_gen(1, use_big_psum=False))
        attn_emit(1, qkproj_gen(2, use_big_psum=False))
        attn_emit(2, qkproj_gen(3, use_big_psum=False))

        # wo reuses wq's slot (wq dead after pair-3 projections)
        wo_sb = shared.tile([P, NPAIR, D], F32R, name="wo_sb", tag="wq_wo")
        for o4 in range(NPAIR):
            nc.sync.dma_start(out=wo_sb[:, o4, :], in_=wor[:, o4, :])

        # pair 3, then the output projection (chunk-granular emission so the
        # scheduler can overlap outproj matmuls with pair-3 tail work)
        attn_emit(3, None)
        for ch in range(NCH):
            outproj_chunk(ch)


def build():
    nc = bacc.Bacc("TRN2", target_bir_lowering=False, debug=False, num_devices=8)
    xT_d = nc.dram_tensor("xT", [D, N], F32R, kind="ExternalInput").ap()
    wq_d = nc.dram_tensor("wq", [D, DLOC], F32R, kind="ExternalInput").ap()
    wk_d = nc.dram_tensor("wk", [D, DLOC], F32R, kind="ExternalInput").ap()
    wv_d = nc.dram_tensor("wv", [D, DLOC], F32R, kind="ExternalInput").ap()
    wo_d = nc.dram_tensor("wo", [DLOC, D], F32R, kind="ExternalInput").ap()
    bq_d = nc.dram_tensor("bq", [P, NPAIR], F32, kind="ExternalInput").ap()
    bk_d = nc.dram_tensor("bk", [P, NPAIR], F32, kind="ExternalInput").ap()
    bv_d = nc.dram_tensor("bv", [P, NPAIR], F32, kind="ExternalInput").ap()
    ones_d = nc.dram_tensor("ones", [P, DH], F32R, kind="ExternalInput").ap()
    out_d = nc.dram_tensor("out", [N, D], F32, kind="ExternalOutput").ap()
    with tile.TileContext(nc) as tc:
        _emit_kernel(tc, xT_d, wq_d, wk_d, wv_d, wo_d, bq_d, bk_d, bv_d, ones_d, out_d)
    nc.compile()
    return nc


_NC = None


def _get_nc():
    global _NC
    if _NC is None:
        _NC = build()
    return _NC


def make_in_maps(x, Wq, bq, Wkv, bkv, Wo, bo):
    x = np.asarray(x, dtype=np.float32)
    Wq = np.asarray(Wq, dtype=np.float32)
    bq = np.asarray(bq, dtype=np.float32)
    Wkv = np.asarray(Wkv, dtype=np.float32)
    bkv = np.asarray(bkv, dtype=np.float32)
    Wo = np.asarray(Wo, dtype=np.float32)

    in_maps = []
    for c in range(8):
        bi, hj = c // 2, c % 2
        sl = slice(hj * DLOC, (hj + 1) * DLOC)
        slv = slice(D + hj * DLOC, D + (hj + 1) * DLOC)
        in_maps.append(
            {
                "xT": np.ascontiguousarray(x[bi].T),
                "wq": np.ascontiguousarray(Wq[:, sl]),
                "wk": np.ascontiguousarray(Wkv[:, sl]),
                "wv": np.ascontiguousarray(Wkv[:, slv]),
                "wo": np.ascontiguousarray(Wo[sl, :]),
                "bq": np.ascontiguousarray(bq[sl].reshape(NPAIR, P).T),
                "bk": np.ascontiguousarray(bkv[sl].reshape(NPAIR, P).T),
                "bv": np.ascontiguousarray(bkv[slv].reshape(NPAIR, P).T),
                "ones": np.ones((P, DH), dtype=np.float32),
            }
        )
    return in_maps


def combine_outputs(results, bo):
    bo = np.asarray(bo, dtype=np.float32)
    outs = [results[c]["out"] for c in range(8)]
    full = np.stack([outs[2 * bi] + outs[2 * bi + 1] for bi in range(4)])
    return (full + bo[None, None, :]).astype(np.float32)


def kernel(x, Wq, bq, Wkv, bkv, Wo, bo, **_ignored):
    nc = _get_nc()
    in_maps = make_in_maps(x, Wq, bq, Wkv, bkv, Wo, bo)
    res = bass_utils.run_bass_kernel_spmd(nc, in_maps, core_ids=list(range(8)))
    return combine_outputs(res.results, bo)
